# revision 3
# baseline (speedup 1.0000x reference)
"""Trainium2 Bass kernel for nn_Attention_3556232921308.

GQA attention layer: RMSNorm -> {Q+gate, K, V} proj -> softmax attention
(no mask, no rope) -> sigmoid output gate -> O proj.
B=2, S=2048, HID=2048, NH=16, NKV=4, HD=128.

Sharding (8 cores): DP over batch (2 groups of 4 cores) x TP over KV heads
(4 ranks per group; each rank owns 1 KV head = 4 Q/gate heads). Gated
attention outputs (bf16) are exchanged with per-head AllGathers; each rank
then computes the O-projection for its quarter of the HID output columns.

v2 changes over the first working version (604us):
 - all activations SBUF-resident (q heads + tanh'd gates persist in SBUF;
   no DRAM roundtrip for q/gate between projection and attention).
 - softmax denominators: instead of one ones-matvec per 128-key tile
   (16 PE matvecs/iter, ~72us of PE), the exp tiles (bf16) are summed with
   a balanced DVE add tree and a single PE matvec per iter finishes the
   128-partition reduction. The matvec weights are 2.0 so ps_sum = 2*sum.
 - sigmoid gate via tanh: sigmoid(g) = (1+tanh(g/2))/2. tanh lives in the
   same ACT table set as exp (no table swap in the hot loop), and is
   precomputed into th_sb during the projection phase where ACT is idle.
   og = ps_o*(1+th) * 1/(2*sum) absorbs both 1/2 factors.
 - reciprocals via reciprocal_approx_fast (custom DVE, ~5x faster) for
   both rstd and the per-iter softmax denominators.
 - attention pt/vnat in bf16 (FWL weight loads; f32r LDW serialization on
   the p@v matmuls cost ~200ns extra per MM).
 - nothing compute-critical is queued on gpsimd behind AllGathers (the
   v1 og-muls and rstd broadcast stalled ~20-30us behind collectives);
   gpsimd runs only the warmup AG, the per-head AGs and the of[] loads.
 - O projection in 3 chunks (heads 0-1 / 2 / 3) accumulated via SBUF so
   the final tail after the last AllGather is only head 3's contraction.
"""
import math
from contextlib import ExitStack

import numpy as np

B, S_FULL, HID = 2, 2048, 2048
NH, NKV, HD = 16, 4, 128
G = NH // NKV  # 4 q heads per kv head = heads per rank
EPS = 1e-6
N_CORES = 8
P = 128
KH = HID // P  # 16 contraction tiles
HQ = HID // 4  # per-rank output column quarter (512)


def build(S=S_FULL):
    import concourse.bass as bass  # noqa: F401
    import concourse.tile as tile
    from concourse import bacc, mybir

    F32R = mybir.dt.float32r
    F32 = mybir.dt.float32
    BF16 = mybir.dt.bfloat16
    AF = mybir.ActivationFunctionType
    ALU = mybir.AluOpType

    SQCH = S // 4  # attention sq chunk (512)
    NW = min(512, S)  # projection free-dim chunk
    NCH = S // NW
    NSK = S // P  # score key tiles (16)
    SCALE = 1.0 / math.sqrt(HD)
    RG = [[0, 1, 2, 3], [4, 5, 6, 7]]

    nc = bacc.Bacc("TRN2", target_bir_lowering=False, debug=False, num_devices=N_CORES)

    hst = nc.declare_dram_parameter("hst", [HID, S], BF16, isOutput=False)
    # weights ship pre-tiled as [P, KH*P] blocks (one linear DMA each)
    wqt = nc.declare_dram_parameter("wqt", [2 * G, P, KH * P], BF16, isOutput=False)
    wkt = nc.declare_dram_parameter("wkt", [P, KH * P], BF16, isOutput=False)
    wvt = nc.declare_dram_parameter("wvt", [P, KH * P], BF16, isOutput=False)
    wot = nc.declare_dram_parameter("wot", [NH * HD, HQ], BF16, isOutput=False)
    onesp = nc.declare_dram_parameter("onesp", [P, 1], F32R, isOutput=False)
    identp = nc.declare_dram_parameter("identp", [P, P], BF16, isOutput=False)
    out = nc.declare_dram_parameter("out", [HQ, S], F32, isOutput=True)

    with tile.TileContext(nc) as tc, ExitStack() as ctx:
        dram = ctx.enter_context(tc.tile_pool(name="dram", bufs=1, space="DRAM"))
        ag_in = [
            dram.tile([P, S], BF16, name=f"ag_in{h}", uniquify=False)
            for h in range(G)
        ]
        ag_out = [
            dram.tile([4 * P, S], BF16, name=f"ag_out{h}", uniquify=False)
            for h in range(G)
        ]
        # tiny warmup collective: absorbs NRT collective-channel init +
        # cross-core launch skew concurrently with the compute phases
        warm_in = dram.tile([P, S // 2], BF16)
        warm_out = dram.tile([4 * P, S // 2], BF16)
        nc.gpsimd.dma_start(out=warm_in[:], in_=hst[0:P, 0:S // 2])
        nc.gpsimd.collective_compute(
            "AllGather",
            mybir.AluOpType.bypass,
            replica_groups=RG,
            ins=[warm_in[:].opt()],
            outs=[warm_out[:].opt()],
        )
        rsd_dram = dram.tile([1, S], F32)
        rs_dram = [
            dram.tile([1, SQCH], F32, name=f"rsd{i}", uniquify=False)
            for i in range(3)
        ]

        consts = ctx.enter_context(tc.tile_pool(name="consts", bufs=1))
        ones_sb = consts.tile([P, 1], F32R)
        nc.gpsimd.dma_start(out=ones_sb[:], in_=onesp[:])
        ident_sb = consts.tile([P, P], BF16)
        nc.gpsimd.dma_start(out=ident_sb[:], in_=identp[:])
        twos_bf = consts.tile([P, 1], BF16)
        nc.vector.memset(twos_bf[:], 2.0)
        eps_t = consts.tile([1, 1], F32)
        nc.vector.memset(eps_t[:], EPS)

        # activations that persist from projection into attention
        keep = ctx.enter_context(tc.tile_pool(name="keep", bufs=1))
        kt_sb = keep.tile([P, S], BF16)
        vnat = [keep.tile([P, P], BF16, name=f"vnat{i}", uniquify=False)
                for i in range(NSK)]
        q_sb = [keep.tile([P, S], BF16, name=f"q{h}", uniquify=False)
                for h in range(G)]
        th_sb = [keep.tile([P, S], BF16, name=f"th{h}", uniquify=False)
                 for h in range(G)]

        # ---- phases 1+2: norm stats + projections (hsT resident) ----
        with ExitStack() as ph12:
            ht_pool = ph12.enter_context(tc.tile_pool(name="ht", bufs=1))
            ht = [ht_pool.tile([P, S], BF16, name=f"ht{k}", uniquify=False)
                  for k in range(KH)]
            vt_sb = ht_pool.tile([P, S], BF16)
            rstd_bc = ht_pool.tile([P, S], F32)
            rb_raw = ht_pool.tile([P, S], F32)

            wkv = ph12.enter_context(tc.tile_pool(name="wkv", bufs=1))
            wk_sb = wkv.tile([P, KH, P], BF16)
            wv_sb = wkv.tile([P, KH, P], BF16)
            nc.sync.dma_start(
                out=wk_sb[:, :, :],
                in_=wkt[:].rearrange("p (k j) -> p k j", k=KH),
            )
            nc.scalar.dma_start(
                out=wv_sb[:, :, :],
                in_=wvt[:].rearrange("p (k j) -> p k j", k=KH),
            )
            for k in range(KH):
                dma_eng = nc.sync if k % 2 == 0 else nc.scalar
                dma_eng.dma_start(out=ht[k][:], in_=hst[k * P:(k + 1) * P, :])

            sqp = ph12.enter_context(tc.tile_pool(name="sqp", bufs=4))
            evp = ph12.enter_context(tc.tile_pool(name="evp", bufs=3))
            wqp = ph12.enter_context(tc.tile_pool(name="wq", bufs=2))

            # phase 1: mean-square stats + K projection chains, software
            # pipelined with the ht DMA stream (K chains + ms matvecs fill
            # the PE while tiles arrive; squares run half on ACT, half DVE)
            with tc.tile_pool(name="msp", bufs=1, space="PSUM") as msp, \
                    tc.tile_pool(name="kps", bufs=1, space="PSUM") as kps:
                ms_ps = [msp.tile([1, NW], F32, name=f"ms{n}", uniquify=False)
                         for n in range(NCH)]
                k_ps = [kps.tile([P, NW], F32, name=f"kp{n}", uniquify=False)
                        for n in range(NCH)]
                sq_prev = None
                for k in range(KH):
                    sq_k = []
                    for n in range(NCH):
                        sqk = sqp.tile([P, NW], F32R)
                        src = ht[k][:, n * NW:(n + 1) * NW]
                        if n % 2 == 0:
                            nc.scalar.activation(sqk[:], src, AF.Square)
                        else:
                            nc.vector.tensor_mul(sqk[:], src, src)
                        sq_k.append(sqk)
                    for n in range(NCH):
                        nc.tensor.matmul(
                            k_ps[n][:],
                            wk_sb[:, k, :],
                            ht[k][:, n * NW:(n + 1) * NW],
                            start=(k == 0),
                            stop=(k == KH - 1),
                        )
                    if sq_prev is not None:
                        for n in range(NCH):
                            nc.tensor.matmul(
                                ms_ps[n][:],
                                ones_sb[:],
                                sq_prev[n][:],
                                start=(k == 1),
                                stop=False,
                            )
                    sq_prev = sq_k
                for n in range(NCH):
                    nc.tensor.matmul(
                        ms_ps[n][:], ones_sb[:], sq_prev[n][:],
                        start=False, stop=True,
                    )
                srow = sqp.tile([1, S], F32, bufs=1)
                for n in range(NCH):
                    nc.scalar.activation(
                        srow[:, n * NW:(n + 1) * NW],
                        ms_ps[n][:],
                        AF.Sqrt,
                        bias=eps_t[:],
                        scale=1.0 / HID,
                    )
                # broadcast rms via a DMA roundtrip on the sync queue (NOT
                # gpsimd: the warmup AllGather holds that queue), then one
                # fast approximate reciprocal on DVE
                nc.sync.dma_start(out=rsd_dram[:, :], in_=srow[:])
                nc.sync.dma_start(
                    out=rb_raw[:], in_=rsd_dram[:, :].to_broadcast((P, S))
                )
                nc.vector.reciprocal_approx_fast(rstd_bc[:], rb_raw[:])
                for n in range(NCH):
                    nc.vector.tensor_mul(
                        kt_sb[:, n * NW:(n + 1) * NW],
                        k_ps[n][:],
                        rstd_bc[:, n * NW:(n + 1) * NW],
                    )

            # V projection + natural-layout transpose
            with tc.tile_pool(name="vps", bufs=2, space="PSUM") as vps:
                for n in range(NCH):
                    ps = vps.tile([P, NW], F32, name="ps_v")
                    for k in range(KH):
                        nc.tensor.matmul(
                            ps[:],
                            wv_sb[:, k, :],
                            ht[k][:, n * NW:(n + 1) * NW],
                            start=(k == 0),
                            stop=(k == KH - 1),
                        )
                    nc.vector.tensor_mul(
                        vt_sb[:, n * NW:(n + 1) * NW],
                        ps[:],
                        rstd_bc[:, n * NW:(n + 1) * NW],
                    )
            with tc.tile_pool(name="tpps", bufs=2, space="PSUM") as tpps:
                for sk in range(NSK):
                    pst = tpps.tile([P, P], BF16)
                    nc.tensor.transpose(
                        pst[:], vt_sb[:, sk * P:(sk + 1) * P], ident_sb[:]
                    )
                    nc.vector.tensor_copy(vnat[sk][:], pst[:])

            # Q + gate projections: q heads scale into q_sb; gate heads
            # scale then tanh(g/2) into th_sb (ACT is idle here, and tanh
            # shares the exp table set so attention never swaps tables)
            with tc.tile_pool(name="qgps", bufs=2, space="PSUM") as qgps:
                def load_wq(m):
                    wq_m = wqp.tile([P, KH, P], BF16, name="wq_m", tag="wq_m")
                    dma_eng = nc.sync if m % 2 == 0 else nc.scalar
                    dma_eng.dma_start(
                        out=wq_m[:, :, :],
                        in_=wqt[m].rearrange("p (k j) -> p k j", k=KH),
                    )
                    return wq_m

                wq_cur = load_wq(0)
                wq_nxt = load_wq(1)
                for m in range(2 * G):
                    for n in range(NCH):
                        ps = qgps.tile([P, NW], F32, name="psqg")
                        for k in range(KH):
                            nc.tensor.matmul(
                                ps[:],
                                wq_cur[:, k, :],
                                ht[k][:, n * NW:(n + 1) * NW],
                                start=(k == 0),
                                stop=(k == KH - 1),
                            )
                        if m < G:
                            nc.vector.tensor_mul(
                                q_sb[m][:, n * NW:(n + 1) * NW],
                                ps[:],
                                rstd_bc[:, n * NW:(n + 1) * NW],
                            )
                        else:
                            gev = evp.tile([P, NW], F32)
                            nc.vector.tensor_mul(
                                gev[:], ps[:],
                                rstd_bc[:, n * NW:(n + 1) * NW],
                            )
                            nc.scalar.activation(
                                th_sb[m - G][:, n * NW:(n + 1) * NW],
                                gev[:],
                                AF.Tanh,
                                scale=0.5,
                            )
                    if m + 2 < 2 * G:
                        wq_cur, wq_nxt = wq_nxt, load_wq(m + 2)
                    else:
                        wq_cur = wq_nxt

        # ---- phases 3+4 pools (allocated in the freed hsT zone) ----
        with ExitStack() as ph34:
            wo_pool = ph34.enter_context(tc.tile_pool(name="wo_bf", bufs=1))
            wo_bf = [wo_pool.tile([P, HQ], BF16, name=f"wo{kf}", uniquify=False)
                     for kf in range(KH)]
            of_pool = ph34.enter_context(tc.tile_pool(name="of", bufs=1))
            of = [of_pool.tile([P, S], BF16, name=f"of{i}", uniquify=False)
                  for i in range(KH)]
            oacc_pool = ph34.enter_context(tc.tile_pool(name="oacc", bufs=1))
            oacc = [oacc_pool.tile([P, NW], F32, name=f"oacc{i}", uniquify=False)
                    for i in range(4 * NCH)]
            for kf in range(KH):
                dma_eng = nc.sync if kf % 2 == 0 else nc.scalar
                dma_eng.dma_start(
                    out=wo_bf[kf][:], in_=wot[kf * P:(kf + 1) * P, :]
                )

            # ---- phase 3: attention ----
            with tc.tile_pool(name="pt", bufs=3) as ptp, tc.tile_pool(
                name="tre", bufs=2
            ) as trp, tc.tile_pool(name="og", bufs=2) as ogp, tc.tile_pool(
                name="sps", bufs=2, space="PSUM"
            ) as sps, tc.tile_pool(
                name="ops", bufs=2, space="PSUM"
            ) as ops, tc.tile_pool(name="sums", bufs=2, space="PSUM") as sums:
                pend_sum = None  # deferred ps_sum matvec (hides the DVE tree tail)

                for h in range(G):
                    for sqc in range(4):
                        ssl = slice(sqc * SQCH, (sqc + 1) * SQCH)
                        qtile = q_sb[h][:, ssl]
                        ps_o = ops.tile([P, SQCH], F32)
                        ps_sum = sums.tile([1, SQCH], F32)

                        # pairs of sk tiles share one 2-bank psum + one exp;
                        # p@v of pair skp-1 is emitted after the scores of
                        # pair skp so the PE never waits on the ACT exp
                        def emit_pv(skp, pt):
                            for j in range(2):
                                sk = 2 * skp + j
                                nc.tensor.matmul(
                                    ps_o[:],
                                    vnat[sk][:],
                                    pt[:, j, :],
                                    start=(sk == 0),
                                    stop=(sk == NSK - 1),
                                )

                        pend_pv = None
                        pts = []
                        l1 = {}
                        for skp in range(NSK // 2):
                            ps_s = sps.tile([P, 2, SQCH], F32)
                            for j in range(2):
                                sk = 2 * skp + j
                                nc.tensor.matmul(
                                    ps_s[:, j, :],
                                    kt_sb[:, sk * P:(sk + 1) * P],
                                    qtile,
                                    start=True,
                                    stop=True,
                                )
                            if skp == 1 and pend_sum is not None:
                                pend_sum()
                                pend_sum = None
                            pt = ptp.tile([P, 2, SQCH], BF16)
                            nc.scalar.activation(pt[:], ps_s[:], AF.Exp, scale=SCALE)
                            if pend_pv is not None:
                                emit_pv(*pend_pv)
                            pend_pv = (skp, pt)
                            pts.append(pt)
                            # balanced bf16 add tree on DVE replaces the 16
                            # per-tile ones-matvecs of v1
                            if skp == 1:
                                l1['a'] = trp.tile([P, 2, SQCH], BF16, name="l1a")
                                nc.vector.tensor_add(l1['a'][:], pts[0][:], pts[1][:])
                            elif skp == 3:
                                l1['b'] = trp.tile([P, 2, SQCH], BF16, name="l1b")
                                nc.vector.tensor_add(l1['b'][:], pts[2][:], pts[3][:])
                                nc.vector.tensor_add(l1['a'][:], l1['a'][:], l1['b'][:])
                            elif skp == 5:
                                l1['c'] = trp.tile([P, 2, SQCH], BF16, name="l1c")
                                nc.vector.tensor_add(l1['c'][:], pts[4][:], pts[5][:])
                            elif skp == 7:
                                l1['d'] = trp.tile([P, 2, SQCH], BF16, name="l1d")
                                nc.vector.tensor_add(l1['d'][:], pts[6][:], pts[7][:])
                                nc.vector.tensor_add(l1['c'][:], l1['c'][:], l1['d'][:])
                                nc.vector.tensor_add(l1['a'][:], l1['a'][:], l1['c'][:])
                        emit_pv(*pend_pv)
                        sfin = trp.tile([P, SQCH], BF16, name="sfin")
                        nc.vector.tensor_add(
                            sfin[:], l1['a'][:, 0, :], l1['a'][:, 1, :]
                        )

                        # denominators: one matvec (weights=2.0 so psum holds
                        # 2*sum), deferred into the next iter's score stream;
                        # then approx-reciprocal + DMA partition-broadcast
                        rs = ogp.tile([1, SQCH], F32)
                        rb = ogp.tile([P, SQCH], F32)
                        rd = rs_dram[(4 * h + sqc) % 3]
                        t1 = ogp.tile([P, SQCH], F32)
                        og = ogp.tile([P, SQCH], BF16)
                        th_tile = th_sb[h][:, ssl]

                        def emit_sum(ps_sum=ps_sum, sfin=sfin, rs=rs, rb=rb,
                                     rd=rd, t1=t1, og=og, ps_o=ps_o,
                                     th_tile=th_tile, h=h, ssl=ssl):
                            nc.tensor.matmul(
                                ps_sum[:], twos_bf[:], sfin[:],
                                start=True, stop=True,
                            )
                            nc.vector.reciprocal_approx_fast(rs[:], ps_sum[:])
                            nc.sync.dma_start(out=rd[:, :], in_=rs[:])
                            nc.sync.dma_start(
                                out=rb[:],
                                in_=rd[:, :].to_broadcast((P, SQCH)),
                            )
                            # gate first: (th+1)*ps_o frees the psum without
                            # waiting on the broadcast DMA roundtrip
                            nc.vector.scalar_tensor_tensor(
                                out=t1[:], in0=th_tile, scalar=1.0,
                                in1=ps_o[:], op0=ALU.add, op1=ALU.mult,
                            )
                            nc.vector.tensor_mul(og[:], t1[:], rb[:])
                            nc.sync.dma_start(out=ag_in[h][:, ssl], in_=og[:])

                        # the deferral may not cross the AllGather below:
                        # AG_h reads ag_in[h] and a later-emitted og write
                        # would order AFTER it (WAR) and feed it stale data
                        if sqc == 3:
                            emit_sum()
                        else:
                            pend_sum = emit_sum

                    nc.gpsimd.collective_compute(
                        "AllGather",
                        mybir.AluOpType.bypass,
                        replica_groups=RG,
                        ins=[ag_in[h][:].opt()],
                        outs=[ag_out[h][:].opt()],
                    )
                    # gpsimd only: these wait on the AllGather and must not
                    # head-of-line-block any compute queue
                    for r in range(4):
                        nc.gpsimd.dma_start(
                            out=of[h * 4 + r][:],
                            in_=ag_out[h][r * P:(r + 1) * P, :],
                        )

            # ---- phase 4: O projection, 3 chunks pipelined with the AGs ----
            with tc.tile_pool(name="outps", bufs=2, space="PSUM") as outps, \
                    tc.tile_pool(name="oev", bufs=3) as oevp:
                NM = HQ // P  # 4
                stages = [(0, 8, 'copy'), (8, 12, 'add'), (12, 16, 'out')]
                for kf0, kf1, mode in stages:
                    for n in range(NCH):
                        pss = [outps.tile([P, NW], F32, name=f"ops{m}")
                               for m in range(NM)]
                        for kf in range(kf0, kf1):
                            for m in range(NM):
                                nc.tensor.matmul(
                                    pss[m][:],
                                    wo_bf[kf][:, m * P:(m + 1) * P],
                                    of[kf][:, n * NW:(n + 1) * NW],
                                    start=(kf == kf0),
                                    stop=(kf == kf1 - 1),
                                )
                        for m in range(NM):
                            acc = oacc[n * NM + m]
                            if mode == 'copy':
                                nc.vector.tensor_copy(acc[:], pss[m][:])
                            elif mode == 'add':
                                nc.vector.tensor_add(acc[:], pss[m][:], acc[:])
                            else:
                                oev = oevp.tile([P, NW], F32)
                                nc.vector.tensor_add(oev[:], pss[m][:], acc[:])
                                dma_eng = nc.sync if (n + m) % 2 == 0 else nc.scalar
                                dma_eng.dma_start(
                                    out=out[m * P:(m + 1) * P,
                                            n * NW:(n + 1) * NW],
                                    in_=oev[:],
                                )

    nc.compile()
    return nc


def make_in_maps(hidden_states, Wq, Wk, Wv, Wo, norm_w, S=S_FULL):
    """Host-side sharding/layout prep. Core c -> (batch c//4, rank c%4)."""
    w1p = (1.0 + norm_w).astype(np.float32)
    WqT = np.ascontiguousarray((Wq * w1p[None, :]).T)  # [HID, 2*NH*HD]
    WkT = np.ascontiguousarray((Wk * w1p[None, :]).T)  # [HID, NKV*HD]
    WvT = np.ascontiguousarray((Wv * w1p[None, :]).T)
    WoT = np.ascontiguousarray(Wo.T)  # [NH*HD, HID]
    # permute feat blocks to match AG stacking: pos h*4+r holds head 4r+h
    perm = [4 * (p % 4) + p // 4 for p in range(NH)]
    WoTp = np.ascontiguousarray(
        WoT.reshape(NH, HD, HID)[perm].reshape(NH * HD, HID)
    )
    ones = np.ones((P, 1), np.float32)
    ident = np.eye(P, dtype=np.float32)

    def tile_w(wt):
        # [HID, C] -> per 128-col block m: [P, KH*P] with wq_m[p, k*P+j] =
        # wt[k*P+p, m*P+j]
        C = wt.shape[1]
        blocks = []
        for m in range(C // P):
            blk = wt[:, m * P:(m + 1) * P].reshape(KH, P, P)
            blocks.append(blk.transpose(1, 0, 2).reshape(P, KH * P))
        return np.ascontiguousarray(np.stack(blocks))

    import ml_dtypes

    bf = ml_dtypes.bfloat16
    in_maps = []
    for c in range(N_CORES):
        b, r = c // 4, c % 4
        qcols = np.r_[r * 512:(r + 1) * 512, NH * HD + r * 512:NH * HD + (r + 1) * 512]
        in_maps.append(
            {
                "hst": np.ascontiguousarray(hidden_states[b, :S].T.astype(bf)),
                "wqt": tile_w(WqT[:, qcols]).astype(bf),
                "wkt": tile_w(WkT[:, r * HD:(r + 1) * HD])[0].astype(bf),
                "wvt": tile_w(WvT[:, r * HD:(r + 1) * HD])[0].astype(bf),
                "wot": np.ascontiguousarray(
                    WoTp[:, r * HQ:(r + 1) * HQ].astype(bf)
                ),
                "onesp": ones,
                "identp": ident.astype(bf),
            }
        )
    return in_maps


def gather_out(results, S=S_FULL):
    out = np.empty((B, S, HID), np.float32)
    for c in range(N_CORES):
        b, r = c // 4, c % 4
        out[b, :, r * HQ:(r + 1) * HQ] = results[c]["out"].T
    return out


_NC_CACHE = {}


def kernel(**inputs) -> np.ndarray:
    from concourse.bass_utils import run_bass_kernel_spmd

    hidden_states = np.asarray(inputs["hidden_states"], dtype=np.float32)
    Wq = np.asarray(inputs["Wq"], dtype=np.float32)
    Wk = np.asarray(inputs["Wk"], dtype=np.float32)
    Wv = np.asarray(inputs["Wv"], dtype=np.float32)
    Wo = np.asarray(inputs["Wo"], dtype=np.float32)
    norm_w = np.asarray(inputs["norm_w"], dtype=np.float32)

    if "nc" not in _NC_CACHE:
        _NC_CACHE["nc"] = build()
    nc = _NC_CACHE["nc"]

    in_maps = make_in_maps(hidden_states, Wq, Wk, Wv, Wo, norm_w)
    res = run_bass_kernel_spmd(nc, in_maps, list(range(N_CORES)))
    return gather_out(res.results)


# revision 4
# speedup vs baseline: 1.2085x; 1.2085x over previous
"""Trainium2 Bass kernel for nn_Attention_3556232921308.

GQA attention layer: RMSNorm -> {Q+gate, K, V} proj -> softmax attention
(no mask, no rope) -> sigmoid output gate -> O proj.
B=2, S=2048, HID=2048, NH=16, NKV=4, HD=128.

Sharding (8 cores): DP over batch (2 groups of 4 cores) x TP over KV heads
(4 ranks per group; each rank owns 1 KV head = 4 Q/gate heads). Gated
attention outputs (bf16) are exchanged with per-head AllGathers; each rank
then computes the O-projection for its quarter of the HID output columns.

v2 changes over the first working version (604us):
 - all activations SBUF-resident (q heads + tanh'd gates persist in SBUF;
   no DRAM roundtrip for q/gate between projection and attention).
 - softmax denominators: instead of one ones-matvec per 128-key tile
   (16 PE matvecs/iter, ~72us of PE), the exp tiles (bf16) are summed with
   a balanced DVE add tree and a single PE matvec per iter finishes the
   128-partition reduction. The matvec weights are 2.0 so ps_sum = 2*sum.
 - sigmoid gate via tanh: sigmoid(g) = (1+tanh(g/2))/2. tanh lives in the
   same ACT table set as exp (no table swap in the hot loop), and is
   precomputed into th_sb during the projection phase where ACT is idle.
   og = ps_o*(1+th) * 1/(2*sum) absorbs both 1/2 factors.
 - reciprocals via reciprocal_approx_fast (custom DVE, ~5x faster) for
   both rstd and the per-iter softmax denominators.
 - attention pt/vnat in bf16 (FWL weight loads; f32r LDW serialization on
   the p@v matmuls cost ~200ns extra per MM).
 - nothing compute-critical is queued on gpsimd behind AllGathers (the
   v1 og-muls and rstd broadcast stalled ~20-30us behind collectives);
   gpsimd runs only the warmup AG, the per-head AGs and the of[] loads.
 - O projection in 3 chunks (heads 0-1 / 2 / 3) accumulated via SBUF so
   the final tail after the last AllGather is only head 3's contraction.
"""
import math
from contextlib import ExitStack

import numpy as np

B, S_FULL, HID = 2, 2048, 2048
NH, NKV, HD = 16, 4, 128
G = NH // NKV  # 4 q heads per kv head = heads per rank
EPS = 1e-6
N_CORES = 8
P = 128
KH = HID // P  # 16 contraction tiles
HQ = HID // 4  # per-rank output column quarter (512)


def build(S=S_FULL):
    import concourse.bass as bass  # noqa: F401
    import concourse.tile as tile
    from concourse import bacc, mybir

    F32R = mybir.dt.float32r
    F32 = mybir.dt.float32
    BF16 = mybir.dt.bfloat16
    AF = mybir.ActivationFunctionType
    ALU = mybir.AluOpType

    SQCH = S // 4  # attention sq chunk (512)
    NW = min(512, S)  # projection free-dim chunk
    NCH = S // NW
    NSK = S // P  # score key tiles (16)
    SCALE = 1.0 / math.sqrt(HD)
    RG = [[0, 1, 2, 3], [4, 5, 6, 7]]

    nc = bacc.Bacc("TRN2", target_bir_lowering=False, debug=False, num_devices=N_CORES)

    hst = nc.declare_dram_parameter("hst", [HID, S], BF16, isOutput=False)
    # weights ship pre-tiled as [P, KH*P] blocks (one linear DMA each)
    wqt = nc.declare_dram_parameter("wqt", [2 * G, P, KH * P], BF16, isOutput=False)
    wkt = nc.declare_dram_parameter("wkt", [P, KH * P], BF16, isOutput=False)
    wvt = nc.declare_dram_parameter("wvt", [P, KH * P], BF16, isOutput=False)
    wot = nc.declare_dram_parameter("wot", [NH * HD, HQ], BF16, isOutput=False)
    onesp = nc.declare_dram_parameter("onesp", [P, 1], F32R, isOutput=False)
    identp = nc.declare_dram_parameter("identp", [P, P], BF16, isOutput=False)
    out = nc.declare_dram_parameter("out", [HQ, S], F32, isOutput=True)

    with tile.TileContext(nc) as tc, ExitStack() as ctx:
        dram = ctx.enter_context(tc.tile_pool(name="dram", bufs=1, space="DRAM"))
        ag_in = [
            dram.tile([P, S], BF16, name=f"ag_in{h}", uniquify=False)
            for h in range(G)
        ]
        ag_out = [
            dram.tile([4 * P, S], BF16, name=f"ag_out{h}", uniquify=False)
            for h in range(G)
        ]
        # tiny warmup collective: absorbs NRT collective-channel init +
        # cross-core launch skew concurrently with the compute phases
        warm_in = dram.tile([P, S // 2], BF16)
        warm_out = dram.tile([4 * P, S // 2], BF16)
        nc.gpsimd.dma_start(out=warm_in[:], in_=hst[0:P, 0:S // 2])
        nc.gpsimd.collective_compute(
            "AllGather",
            mybir.AluOpType.bypass,
            replica_groups=RG,
            ins=[warm_in[:].opt()],
            outs=[warm_out[:].opt()],
        )
        rsd_dram = dram.tile([1, S], F32)
        rs_dram = [
            dram.tile([1, SQCH], F32, name=f"rsd{i}", uniquify=False)
            for i in range(3)
        ]

        consts = ctx.enter_context(tc.tile_pool(name="consts", bufs=1))
        ones_sb = consts.tile([P, 1], F32R)
        nc.gpsimd.dma_start(out=ones_sb[:], in_=onesp[:])
        ident_sb = consts.tile([P, P], BF16)
        nc.gpsimd.dma_start(out=ident_sb[:], in_=identp[:])
        twos_bf = consts.tile([P, 1], BF16)
        nc.vector.memset(twos_bf[:], 2.0)
        eps_t = consts.tile([1, 1], F32)
        nc.vector.memset(eps_t[:], EPS)

        # activations that persist from projection into attention
        keep = ctx.enter_context(tc.tile_pool(name="keep", bufs=1))
        kt_sb = keep.tile([P, S], BF16)
        vnat = [keep.tile([P, P], BF16, name=f"vnat{i}", uniquify=False)
                for i in range(NSK)]
        q_sb = [keep.tile([P, S], BF16, name=f"q{h}", uniquify=False)
                for h in range(G)]
        th_sb = [keep.tile([P, S], BF16, name=f"th{h}", uniquify=False)
                 for h in range(G)]

        # ---- phases 1+2: norm stats + projections (hsT resident) ----
        with ExitStack() as ph12:
            ht_pool = ph12.enter_context(tc.tile_pool(name="ht", bufs=1))
            ht = [ht_pool.tile([P, S], BF16, name=f"ht{k}", uniquify=False)
                  for k in range(KH)]
            vt_sb = ht_pool.tile([P, S], BF16)
            rstd_bc = ht_pool.tile([P, S], F32)
            rb_raw = ht_pool.tile([P, S], F32)

            wkv = ph12.enter_context(tc.tile_pool(name="wkv", bufs=1))
            wk_sb = wkv.tile([P, KH, P], BF16)
            wv_sb = wkv.tile([P, KH, P], BF16)
            nc.sync.dma_start(
                out=wk_sb[:, :, :],
                in_=wkt[:].rearrange("p (k j) -> p k j", k=KH),
            )
            nc.scalar.dma_start(
                out=wv_sb[:, :, :],
                in_=wvt[:].rearrange("p (k j) -> p k j", k=KH),
            )
            for k in range(KH):
                dma_eng = nc.sync if k % 2 == 0 else nc.scalar
                dma_eng.dma_start(out=ht[k][:], in_=hst[k * P:(k + 1) * P, :])

            sqp = ph12.enter_context(tc.tile_pool(name="sqp", bufs=4))
            evp = ph12.enter_context(tc.tile_pool(name="evp", bufs=3))
            wqp = ph12.enter_context(tc.tile_pool(name="wq", bufs=2))

            # phase 1: mean-square stats + K projection chains, software
            # pipelined with the ht DMA stream (K chains + ms matvecs fill
            # the PE while tiles arrive; squares run half on ACT, half DVE)
            with tc.tile_pool(name="msp", bufs=1, space="PSUM") as msp, \
                    tc.tile_pool(name="kps", bufs=1, space="PSUM") as kps:
                ms_ps = [msp.tile([1, NW], F32, name=f"ms{n}", uniquify=False)
                         for n in range(NCH)]
                k_ps = [kps.tile([P, NW], F32, name=f"kp{n}", uniquify=False)
                        for n in range(NCH)]
                sq_prev = None
                for k in range(KH):
                    sq_k = []
                    for n in range(NCH):
                        sqk = sqp.tile([P, NW], F32R)
                        src = ht[k][:, n * NW:(n + 1) * NW]
                        if n % 2 == 0:
                            nc.scalar.activation(sqk[:], src, AF.Square)
                        else:
                            nc.vector.tensor_mul(sqk[:], src, src)
                        sq_k.append(sqk)
                    for n in range(NCH):
                        nc.tensor.matmul(
                            k_ps[n][:],
                            wk_sb[:, k, :],
                            ht[k][:, n * NW:(n + 1) * NW],
                            start=(k == 0),
                            stop=(k == KH - 1),
                        )
                    if sq_prev is not None:
                        for n in range(NCH):
                            nc.tensor.matmul(
                                ms_ps[n][:],
                                ones_sb[:],
                                sq_prev[n][:],
                                start=(k == 1),
                                stop=False,
                            )
                    sq_prev = sq_k
                for n in range(NCH):
                    nc.tensor.matmul(
                        ms_ps[n][:], ones_sb[:], sq_prev[n][:],
                        start=False, stop=True,
                    )
                srow = sqp.tile([1, S], F32, bufs=1)
                for n in range(NCH):
                    nc.scalar.activation(
                        srow[:, n * NW:(n + 1) * NW],
                        ms_ps[n][:],
                        AF.Sqrt,
                        bias=eps_t[:],
                        scale=1.0 / HID,
                    )
                # broadcast rms via a DMA roundtrip on the sync queue (NOT
                # gpsimd: the warmup AllGather holds that queue), then one
                # fast approximate reciprocal on DVE
                nc.sync.dma_start(out=rsd_dram[:, :], in_=srow[:])
                nc.sync.dma_start(
                    out=rb_raw[:], in_=rsd_dram[:, :].to_broadcast((P, S))
                )
                nc.vector.reciprocal_approx_fast(rstd_bc[:], rb_raw[:])
                for n in range(NCH):
                    nc.vector.tensor_mul(
                        kt_sb[:, n * NW:(n + 1) * NW],
                        k_ps[n][:],
                        rstd_bc[:, n * NW:(n + 1) * NW],
                    )

            # V projection + natural-layout transpose
            with tc.tile_pool(name="vps", bufs=2, space="PSUM") as vps:
                for n in range(NCH):
                    ps = vps.tile([P, NW], F32, name="ps_v")
                    for k in range(KH):
                        nc.tensor.matmul(
                            ps[:],
                            wv_sb[:, k, :],
                            ht[k][:, n * NW:(n + 1) * NW],
                            start=(k == 0),
                            stop=(k == KH - 1),
                        )
                    nc.vector.tensor_mul(
                        vt_sb[:, n * NW:(n + 1) * NW],
                        ps[:],
                        rstd_bc[:, n * NW:(n + 1) * NW],
                    )
            with tc.tile_pool(name="tpps", bufs=2, space="PSUM") as tpps:
                for sk in range(NSK):
                    pst = tpps.tile([P, P], BF16)
                    nc.tensor.transpose(
                        pst[:], vt_sb[:, sk * P:(sk + 1) * P], ident_sb[:]
                    )
                    nc.vector.tensor_copy(vnat[sk][:], pst[:])

            # Q + gate projections: q heads scale into q_sb; gate heads
            # scale then tanh(g/2) into th_sb (ACT is idle here, and tanh
            # shares the exp table set so attention never swaps tables)
            with tc.tile_pool(name="qgps", bufs=2, space="PSUM") as qgps:
                def load_wq(m):
                    wq_m = wqp.tile([P, KH, P], BF16, name="wq_m", tag="wq_m")
                    dma_eng = nc.sync if m % 2 == 0 else nc.scalar
                    dma_eng.dma_start(
                        out=wq_m[:, :, :],
                        in_=wqt[m].rearrange("p (k j) -> p k j", k=KH),
                    )
                    return wq_m

                wq_cur = load_wq(0)
                wq_nxt = load_wq(1)
                for m in range(2 * G):
                    for n in range(NCH):
                        ps = qgps.tile([P, NW], F32, name="psqg")
                        for k in range(KH):
                            nc.tensor.matmul(
                                ps[:],
                                wq_cur[:, k, :],
                                ht[k][:, n * NW:(n + 1) * NW],
                                start=(k == 0),
                                stop=(k == KH - 1),
                            )
                        if m < G:
                            nc.vector.tensor_mul(
                                q_sb[m][:, n * NW:(n + 1) * NW],
                                ps[:],
                                rstd_bc[:, n * NW:(n + 1) * NW],
                            )
                        else:
                            gev = evp.tile([P, NW], F32)
                            nc.vector.tensor_mul(
                                gev[:], ps[:],
                                rstd_bc[:, n * NW:(n + 1) * NW],
                            )
                            nc.scalar.activation(
                                th_sb[m - G][:, n * NW:(n + 1) * NW],
                                gev[:],
                                AF.Tanh,
                                scale=0.5,
                            )
                    if m + 2 < 2 * G:
                        wq_cur, wq_nxt = wq_nxt, load_wq(m + 2)
                    else:
                        wq_cur = wq_nxt

        # ---- phases 3+4 pools (allocated in the freed hsT zone) ----
        with ExitStack() as ph34:
            wo_pool = ph34.enter_context(tc.tile_pool(name="wo_bf", bufs=1))
            wo_bf = [wo_pool.tile([P, HQ], BF16, name=f"wo{kf}", uniquify=False)
                     for kf in range(KH)]
            of_pool = ph34.enter_context(tc.tile_pool(name="of", bufs=1))
            of = [of_pool.tile([P, S], BF16, name=f"of{i}", uniquify=False)
                  for i in range(KH)]
            oacc_pool = ph34.enter_context(tc.tile_pool(name="oacc", bufs=1))
            oacc = [oacc_pool.tile([P, NW], F32, name=f"oacc{i}", uniquify=False)
                    for i in range(4 * NCH)]
            for kf in range(KH):
                dma_eng = nc.sync if kf % 2 == 0 else nc.scalar
                dma_eng.dma_start(
                    out=wo_bf[kf][:], in_=wot[kf * P:(kf + 1) * P, :]
                )

            # ---- phase 3: attention ----
            with tc.tile_pool(name="pt", bufs=3) as ptp, tc.tile_pool(
                name="tre", bufs=2
            ) as trp, tc.tile_pool(name="og", bufs=2) as ogp, tc.tile_pool(
                name="sps", bufs=2, space="PSUM"
            ) as sps, tc.tile_pool(
                name="ops", bufs=2, space="PSUM"
            ) as ops, tc.tile_pool(name="sums", bufs=2, space="PSUM") as sums:
                pend_sum = None  # deferred ps_sum matvec (hides the DVE tree tail)

                for h in range(G):
                    for sqc in range(4):
                        ssl = slice(sqc * SQCH, (sqc + 1) * SQCH)
                        qtile = q_sb[h][:, ssl]
                        ps_o = ops.tile([P, SQCH], F32)
                        ps_sum = sums.tile([1, SQCH], F32)

                        # pairs of sk tiles share one 2-bank psum + one exp;
                        # p@v of pair skp-1 is emitted after the scores of
                        # pair skp so the PE never waits on the ACT exp
                        def emit_pv(skp, pt):
                            for j in range(2):
                                sk = 2 * skp + j
                                nc.tensor.matmul(
                                    ps_o[:],
                                    vnat[sk][:],
                                    pt[:, j, :],
                                    start=(sk == 0),
                                    stop=(sk == NSK - 1),
                                )

                        pend_pv = None
                        pts = []
                        l1 = {}
                        for skp in range(NSK // 2):
                            ps_s = sps.tile([P, 2, SQCH], F32)
                            for j in range(2):
                                sk = 2 * skp + j
                                nc.tensor.matmul(
                                    ps_s[:, j, :],
                                    kt_sb[:, sk * P:(sk + 1) * P],
                                    qtile,
                                    start=True,
                                    stop=True,
                                )
                            if skp == 1 and pend_sum is not None:
                                pend_sum()
                                pend_sum = None
                            pt = ptp.tile([P, 2, SQCH], BF16)
                            nc.scalar.activation(pt[:], ps_s[:], AF.Exp, scale=SCALE)
                            if pend_pv is not None:
                                emit_pv(*pend_pv)
                            pend_pv = (skp, pt)
                            pts.append(pt)
                            # balanced bf16 add tree on DVE replaces the 16
                            # per-tile ones-matvecs of v1. Flat 2D views: the
                            # DVE 2x uop mode needs dense step-1 APs.
                            def flat(t):
                                return t[:, :, :].rearrange("p a b -> p (a b)")

                            if skp == 1:
                                l1['a'] = trp.tile([P, 2, SQCH], BF16, name="l1a")
                                nc.vector.tensor_add(
                                    flat(l1['a']), flat(pts[0]), flat(pts[1]))
                            elif skp == 3:
                                l1['b'] = trp.tile([P, 2, SQCH], BF16, name="l1b")
                                nc.vector.tensor_add(
                                    flat(l1['b']), flat(pts[2]), flat(pts[3]))
                                nc.vector.tensor_add(
                                    flat(l1['a']), flat(l1['a']), flat(l1['b']))
                            elif skp == 5:
                                l1['c'] = trp.tile([P, 2, SQCH], BF16, name="l1c")
                                nc.vector.tensor_add(
                                    flat(l1['c']), flat(pts[4]), flat(pts[5]))
                            elif skp == 7:
                                l1['d'] = trp.tile([P, 2, SQCH], BF16, name="l1d")
                                nc.vector.tensor_add(
                                    flat(l1['d']), flat(pts[6]), flat(pts[7]))
                                nc.vector.tensor_add(
                                    flat(l1['c']), flat(l1['c']), flat(l1['d']))
                                nc.vector.tensor_add(
                                    flat(l1['a']), flat(l1['a']), flat(l1['c']))
                        emit_pv(*pend_pv)
                        sfin = trp.tile([P, SQCH], BF16, name="sfin")
                        nc.vector.tensor_add(
                            sfin[:], l1['a'][:, 0, :], l1['a'][:, 1, :]
                        )

                        # denominators: one matvec (weights=2.0 so psum holds
                        # 2*sum), deferred into the next iter's score stream;
                        # then approx-reciprocal + DMA partition-broadcast
                        rs = ogp.tile([1, SQCH], F32)
                        rb = ogp.tile([P, SQCH], F32)
                        rd = rs_dram[(4 * h + sqc) % 3]
                        t1 = ogp.tile([P, SQCH], F32)
                        og = ogp.tile([P, SQCH], BF16)
                        th_tile = th_sb[h][:, ssl]

                        def emit_sum(ps_sum=ps_sum, sfin=sfin, rs=rs, rb=rb,
                                     rd=rd, t1=t1, og=og, ps_o=ps_o,
                                     th_tile=th_tile, h=h, ssl=ssl):
                            nc.tensor.matmul(
                                ps_sum[:], twos_bf[:], sfin[:],
                                start=True, stop=True,
                            )
                            nc.vector.reciprocal_approx_fast(rs[:], ps_sum[:])
                            nc.sync.dma_start(out=rd[:, :], in_=rs[:])
                            nc.sync.dma_start(
                                out=rb[:],
                                in_=rd[:, :].to_broadcast((P, SQCH)),
                            )
                            # gate first: (th+1)*ps_o frees the psum without
                            # waiting on the broadcast DMA roundtrip
                            nc.vector.scalar_tensor_tensor(
                                out=t1[:], in0=th_tile, scalar=1.0,
                                in1=ps_o[:], op0=ALU.add, op1=ALU.mult,
                            )
                            nc.vector.tensor_mul(og[:], t1[:], rb[:])
                            nc.sync.dma_start(out=ag_in[h][:, ssl], in_=og[:])

                        # the deferral may not cross the AllGather below:
                        # AG_h reads ag_in[h] and a later-emitted og write
                        # would order AFTER it (WAR) and feed it stale data
                        if sqc == 3:
                            emit_sum()
                        else:
                            pend_sum = emit_sum

                    nc.gpsimd.collective_compute(
                        "AllGather",
                        mybir.AluOpType.bypass,
                        replica_groups=RG,
                        ins=[ag_in[h][:].opt()],
                        outs=[ag_out[h][:].opt()],
                    )
                    # gpsimd only: these wait on the AllGather and must not
                    # head-of-line-block any compute queue
                    for r in range(4):
                        nc.gpsimd.dma_start(
                            out=of[h * 4 + r][:],
                            in_=ag_out[h][r * P:(r + 1) * P, :],
                        )

            # ---- phase 4: O projection, 3 chunks pipelined with the AGs ----
            with tc.tile_pool(name="outps", bufs=2, space="PSUM") as outps, \
                    tc.tile_pool(name="oev", bufs=3) as oevp:
                NM = HQ // P  # 4
                stages = [(0, 8, 'copy'), (8, 12, 'add'), (12, 16, 'out')]
                for kf0, kf1, mode in stages:
                    for n in range(NCH):
                        pss = [outps.tile([P, NW], F32, name=f"ops{m}")
                               for m in range(NM)]
                        for kf in range(kf0, kf1):
                            for m in range(NM):
                                nc.tensor.matmul(
                                    pss[m][:],
                                    wo_bf[kf][:, m * P:(m + 1) * P],
                                    of[kf][:, n * NW:(n + 1) * NW],
                                    start=(kf == kf0),
                                    stop=(kf == kf1 - 1),
                                )
                        for m in range(NM):
                            acc = oacc[n * NM + m]
                            if mode == 'copy':
                                nc.vector.tensor_copy(acc[:], pss[m][:])
                            elif mode == 'add':
                                nc.vector.tensor_add(acc[:], pss[m][:], acc[:])
                            else:
                                oev = oevp.tile([P, NW], F32)
                                nc.vector.tensor_add(oev[:], pss[m][:], acc[:])
                                dma_eng = nc.sync if (n + m) % 2 == 0 else nc.scalar
                                dma_eng.dma_start(
                                    out=out[m * P:(m + 1) * P,
                                            n * NW:(n + 1) * NW],
                                    in_=oev[:],
                                )

    nc.compile()
    return nc


def make_in_maps(hidden_states, Wq, Wk, Wv, Wo, norm_w, S=S_FULL):
    """Host-side sharding/layout prep. Core c -> (batch c//4, rank c%4)."""
    w1p = (1.0 + norm_w).astype(np.float32)
    WqT = np.ascontiguousarray((Wq * w1p[None, :]).T)  # [HID, 2*NH*HD]
    WkT = np.ascontiguousarray((Wk * w1p[None, :]).T)  # [HID, NKV*HD]
    WvT = np.ascontiguousarray((Wv * w1p[None, :]).T)
    WoT = np.ascontiguousarray(Wo.T)  # [NH*HD, HID]
    # permute feat blocks to match AG stacking: pos h*4+r holds head 4r+h
    perm = [4 * (p % 4) + p // 4 for p in range(NH)]
    WoTp = np.ascontiguousarray(
        WoT.reshape(NH, HD, HID)[perm].reshape(NH * HD, HID)
    )
    ones = np.ones((P, 1), np.float32)
    ident = np.eye(P, dtype=np.float32)

    def tile_w(wt):
        # [HID, C] -> per 128-col block m: [P, KH*P] with wq_m[p, k*P+j] =
        # wt[k*P+p, m*P+j]
        C = wt.shape[1]
        blocks = []
        for m in range(C // P):
            blk = wt[:, m * P:(m + 1) * P].reshape(KH, P, P)
            blocks.append(blk.transpose(1, 0, 2).reshape(P, KH * P))
        return np.ascontiguousarray(np.stack(blocks))

    import ml_dtypes

    bf = ml_dtypes.bfloat16
    in_maps = []
    for c in range(N_CORES):
        b, r = c // 4, c % 4
        qcols = np.r_[r * 512:(r + 1) * 512, NH * HD + r * 512:NH * HD + (r + 1) * 512]
        in_maps.append(
            {
                "hst": np.ascontiguousarray(hidden_states[b, :S].T.astype(bf)),
                "wqt": tile_w(WqT[:, qcols]).astype(bf),
                "wkt": tile_w(WkT[:, r * HD:(r + 1) * HD])[0].astype(bf),
                "wvt": tile_w(WvT[:, r * HD:(r + 1) * HD])[0].astype(bf),
                "wot": np.ascontiguousarray(
                    WoTp[:, r * HQ:(r + 1) * HQ].astype(bf)
                ),
                "onesp": ones,
                "identp": ident.astype(bf),
            }
        )
    return in_maps


def gather_out(results, S=S_FULL):
    out = np.empty((B, S, HID), np.float32)
    for c in range(N_CORES):
        b, r = c // 4, c % 4
        out[b, :, r * HQ:(r + 1) * HQ] = results[c]["out"].T
    return out


_NC_CACHE = {}


def kernel(**inputs) -> np.ndarray:
    from concourse.bass_utils import run_bass_kernel_spmd

    hidden_states = np.asarray(inputs["hidden_states"], dtype=np.float32)
    Wq = np.asarray(inputs["Wq"], dtype=np.float32)
    Wk = np.asarray(inputs["Wk"], dtype=np.float32)
    Wv = np.asarray(inputs["Wv"], dtype=np.float32)
    Wo = np.asarray(inputs["Wo"], dtype=np.float32)
    norm_w = np.asarray(inputs["norm_w"], dtype=np.float32)

    if "nc" not in _NC_CACHE:
        _NC_CACHE["nc"] = build()
    nc = _NC_CACHE["nc"]

    in_maps = make_in_maps(hidden_states, Wq, Wk, Wv, Wo, norm_w)
    res = run_bass_kernel_spmd(nc, in_maps, list(range(N_CORES)))
    return gather_out(res.results)
